# revision 22
# baseline (speedup 1.0000x reference)
"""Trainium2 Bass kernel for differentiable KDE (Gaussian kernel density).

Math (h = 1, C = 0.5/sqrt(2*pi)):
    density[i] = mean_j exp(-C * ||x_i - d_j||^2)
               = sum_j exp(2C x_i.d_j - C||d_j||^2 - C||x_i||^2 - lnM)

Sharding: data-parallel over x rows (1024 per core), data replicated.

Per-core architecture (i = x row as PSUM partition, j = data row as free dim):
    - ACT (scalar) is the hard floor: 8.39M exps at 1 elem/cycle/lane
      @1.2GHz ~= 55us. Everything else is kept below it and overlapped.
    - Data path: DMA f32 rows -> DVE cast bf16 -> DMA to DRAM scratch ->
      DMA-xbar-transpose back as dataT [128(d), M] bf16 (no PSUM, no PE).
      Norms come from the same bf16 values, so the -C||d_j||^2 term
      matches the quantized data exactly; same for x (bf16 + matching
      norm bias), making each kernel term exact for the rounded points.
    - Main loop, per (j-superblock of 2048) x (i-block of 128):
      PE: 4x matmul psum[128, 512] = xT_b.T @ dataT (bf16) plus a rank-1
      fp16 matmul (ones x dnrow) accumulating -C||d_j||^2 into psum;
      ACT: e = exp(2C*psum + bias_i) -> bf16 (bias = -C||x_i||^2 - lnM);
      DVE: tensor_scalar (mult 1, add 0) with accum_out -> per-partition
      running sums at 4x DVE rate.
    - PSUM: 2 x [128, 2048] main tiles = all 8 banks, double-buffered.
"""
import math
from contextlib import ExitStack

import numpy as np

from concourse import bacc, mybir, tile
from concourse.bass_utils import run_bass_kernel_spmd
from concourse import masks

N, M, D = 8192, 8192, 128
NCORES = 8
NS = N // NCORES            # 1024 x-rows per core
P = 128                     # partitions
NB = NS // P                # 8 i-blocks
NCH = 8                     # data chunks (1024 rows each)
RPC = M // NCH              # 1024 rows per chunk
RPP = RPC // P              # 8 rows per partition per chunk
NJS = 4                     # j-superblocks
JW = M // NJS               # 2048 j per superblock

C = 0.5 / math.sqrt(2.0 * math.pi)
TWO_C = 2.0 * C
LNM = math.log(float(M))

F32 = mybir.dt.float32
F32R = mybir.dt.float32r
BF16 = mybir.dt.bfloat16
FP16 = mybir.dt.float16

_CACHED_NC = None


def _build():
    nc = bacc.Bacc("TRN2", target_bir_lowering=False, debug=False)
    x_d = nc.dram_tensor("x", [NS, D], F32, kind="ExternalInput")
    d_d = nc.dram_tensor("data", [M, D], F32, kind="ExternalInput")
    o_d = nc.dram_tensor("out", [NS, 1], F32, kind="ExternalOutput")
    dbf_d = nc.dram_tensor("dbf", [M, D], BF16, kind="Internal")
    nscr_d = nc.dram_tensor("nscr", [M], FP16, kind="Internal")

    # row p*RPP + r lands at [p, r, :]: 4KB contiguous per partition
    x_re = x_d.ap().rearrange("(p r) d -> p r d", p=P)          # [128, 8, 128]
    d_re = d_d.ap().rearrange("(c p r) d -> c p r d", c=NCH, p=P)
    dbf_re = dbf_d.ap().rearrange("(c p r) d -> c p r d", c=NCH, p=P)

    with tile.TileContext(nc) as tc, ExitStack() as ctx:
        const_pool = ctx.enter_context(tc.tile_pool(name="const", bufs=1))
        big_pool = ctx.enter_context(tc.tile_pool(name="big", bufs=1))
        drow_pool = ctx.enter_context(tc.tile_pool(name="drow", bufs=3))
        dbf_pool = ctx.enter_context(tc.tile_pool(name="dbf", bufs=2))
        gsq_pool = ctx.enter_context(tc.tile_pool(name="gsq", bufs=2))
        ps_main = ctx.enter_context(tc.tile_pool(name="psm", bufs=2, space="PSUM"))

        ident = const_pool.tile([P, P], F32, tag="ident")
        masks.make_identity(nc, ident[:])
        ones16 = const_pool.tile([1, P], FP16, tag="ones16")
        nc.gpsimd.memset(ones16[:], 1.0)

        xrow = big_pool.tile([P, NB, P], F32, tag="xrow")
        xbf = big_pool.tile([P, NB, P], BF16, tag="xbf")
        xT = big_pool.tile([P, NS], BF16, tag="xT")
        xsqs = big_pool.tile([P, NB * P], F32, tag="xsqs")
        xnsq = big_pool.tile([P, NB], F32, tag="xnsq")
        xbias = big_pool.tile([P, NB], F32, tag="xbias")
        dataT = big_pool.tile([P, M], BF16, tag="dataT")
        dnsq = big_pool.tile([P, NCH * RPP], F32, tag="dnsq")
        dnst = big_pool.tile([P, RPP], FP16, tag="dnst")
        dnrow = big_pool.tile([1, M], FP16, tag="dnrow")
        pacc = big_pool.tile([P, NB * NJS], F32, tag="pacc")
        outsb = big_pool.tile([P, NB], F32, tag="outsb")
        # e is write-only scratch (ACT's accumulator carries the reduction)
        escr = big_pool.tile([P, JW], BF16, tag="escr")

        # ---- x prologue ----
        nc.sync.dma_start(xrow[:], x_re)
        # bf16-quantize x; norms use the same quantized values as the matmul
        nc.vector.tensor_copy(xbf[:], xrow[:])
        nc.vector.tensor_mul(
            xsqs[:], xbf[:].rearrange("p a b -> p (a b)"),
            xbf[:].rearrange("p a b -> p (a b)"))
        nc.vector.tensor_reduce(
            xnsq[:].rearrange("p (r o) -> p r o", o=1),
            xsqs[:].rearrange("p (r d) -> p r d", d=P),
            axis=mybir.AxisListType.X, op=mybir.AluOpType.add)
        nc.gpsimd.tensor_scalar_mul(xbias[:], xnsq[:], -C)
        nc.gpsimd.tensor_scalar_add(xbias[:], xbias[:], -LNM)
        # x transposes through one main psum tile (before main loop needs it)
        pmx = ps_main.tile([P, JW], F32, tag="pm")
        for t in range(NB):
            nc.tensor.transpose(pmx[:, t * P:(t + 1) * P], xrow[:, t, :],
                                ident[:])
        nc.vector.tensor_copy(xT[:], pmx[:, 0:NS])
        # PE pre-warm: sustained dummy matmuls force the HAM clock-gate to
        # 2.4GHz before the main loop; results are never read
        pmw = ps_main.tile([P, JW], F32, tag="pm")
        for w in range(4):
            nc.tensor.matmul(pmw[:, (w % 2) * 512:(w % 2) * 512 + 512],
                             ident[:], xsqs[:, (w % 2) * 512:(w % 2) * 512 + 512],
                             start=True, stop=True)

        # ---- streamed data prologue + main loop ----
        for ch in range(NCH):
            rsl = slice(ch * RPC, (ch + 1) * RPC)
            drow = drow_pool.tile([P, RPP, P], F32, tag="drow")
            nc.sync.dma_start(drow[:], d_re[ch])
            dbf = dbf_pool.tile([P, RPP, P], BF16, tag="dbf")
            nc.vector.tensor_copy(dbf[:], drow[:])
            # bf16 rows -> DRAM -> xbar-transposed back (same queue = ordered)
            nc.sync.dma_start(dbf_re[ch], dbf[:])
            nc.sync.dma_start_transpose(dataT[:, rsl], dbf_d.ap()[rsl, :])
            # norms from the same bf16 values the matmul will see
            g = gsq_pool.tile([P, RPC], F32, tag="gsq")
            dbf_f = dbf[:].rearrange("p a b -> p (a b)")
            nc.gpsimd.tensor_mul(g[:], dbf_f, dbf_f)
            csl = slice(ch * RPP, (ch + 1) * RPP)
            nc.vector.tensor_reduce(
                dnsq[:, csl].rearrange("p (r o) -> p r o", o=1),
                g[:].rearrange("p (r d) -> p r d", d=P),
                axis=mybir.AxisListType.X, op=mybir.AluOpType.add)
            # -||d_j||^2/2 as fp16 row in j order (ACT applies the 2C scale
            # to the whole psum, giving -C||d_j||^2): p-major flat via DRAM
            nc.gpsimd.tensor_scalar_mul(dnst[:], dnsq[:, csl], -0.5)
            nc.gpsimd.dma_start(
                nscr_d.ap()[rsl].rearrange("(p r) -> p r", p=P), dnst[:])
            nc.gpsimd.dma_start(
                dnrow[:, rsl],
                nscr_d.ap()[rsl].rearrange("(o q) -> o q", o=1))

            if ch % 2 == 1:
                js = ch // 2
                for b in range(NB):
                    pm = ps_main.tile([P, JW], F32, tag="pm")
                    lhs = xT[:, b * P:(b + 1) * P]
                    # grouped by stationary operand: 4x rank-1 bias, then
                    # 4x main, so weights swap twice per tile, not 8 times
                    for q in range(4):
                        qsl = slice(q * 512, (q + 1) * 512)
                        jsl = slice(js * JW + q * 512, js * JW + (q + 1) * 512)
                        nc.tensor.matmul(pm[:, qsl], ones16[:], dnrow[:, jsl],
                                         start=True, stop=False)
                    for q in range(4):
                        qsl = slice(q * 512, (q + 1) * 512)
                        jsl = slice(js * JW + q * 512, js * JW + (q + 1) * 512)
                        nc.tensor.matmul(pm[:, qsl], lhs, dataT[:, jsl],
                                         start=False, stop=True)
                    nc.scalar.activation(
                        escr[:], pm[:], mybir.ActivationFunctionType.Exp,
                        bias=xbias[:, b:b + 1], scale=TWO_C,
                        accum_out=pacc[:, b * NJS + js:b * NJS + js + 1])

        # ---- epilogue: density row p*8+b at outsb[p, b] ----
        for b in range(NB):
            nc.vector.tensor_reduce(
                outsb[:, b:b + 1], pacc[:, b * NJS:(b + 1) * NJS],
                axis=mybir.AxisListType.X, op=mybir.AluOpType.add)
        nc.sync.dma_start(
            o_d.ap().rearrange("(p r) o -> p (r o)", p=P), outsb[:])

    nc.compile()
    return nc


def kernel(x, data):
    global _CACHED_NC
    x = np.ascontiguousarray(np.asarray(x, dtype=np.float32))
    data = np.ascontiguousarray(np.asarray(data, dtype=np.float32))
    assert x.shape == (N, D) and data.shape == (M, D)

    if _CACHED_NC is None:
        _CACHED_NC = _build()
    nc = _CACHED_NC

    in_maps = [
        {"x": x[c * NS:(c + 1) * NS], "data": data} for c in range(NCORES)
    ]
    res = run_bass_kernel_spmd(nc, in_maps, list(range(NCORES)))
    dens = np.concatenate(
        [np.asarray(res.results[c]["out"]).reshape(NS) for c in range(NCORES)]
    )
    return dens.reshape(N, 1).astype(np.float32)


if __name__ == "__main__":
    rng = np.random.default_rng(0)
    x = rng.standard_normal((N, D), dtype=np.float32)
    data = rng.standard_normal((M, D), dtype=np.float32)
    out = kernel(x, data)
    print("kernel out", out.shape, out[:4, 0])


# revision 24
# speedup vs baseline: 1.3437x; 1.3437x over previous
"""Trainium2 Bass kernel for differentiable KDE (Gaussian kernel density estimate).

Math (h = 1):
    sq[i,j]    = ||x_i||^2 + ||d_j||^2 - 2 x_i.d_j
    density[i] = mean_j exp(-C * sq[i,j]),   C = 0.5 / sqrt(2*pi)
               = exp(-C||x_i||^2 - ln M) * sum_j exp(2C x_i.d_j - C||d_j||^2)

Sharding: data-parallel over x rows (1024 per core), data replicated.

Per-core pipeline (j = data row as PSUM partition, i = x row as free dim):
    - DMA data in 8 row-interleaved chunks; per 128-row tile: DVE
      square+reduce -> ||d_j||^2 bias column, PE transpose -> dataT in SBUF
      as float32r (tf32-grade matmul dtype, full-rate weight streaming).
    - Main loop over 64 j-tiles: PE matmul psum[j=128, i=1024] =
      dataT_jt.T @ xT (float32r), ACT exp with per-partition bias
      -C||d_j||^2 and scale 2C -> E (float32r), PE matvec with all-ones
      stationary accumulates sum_j E over all 64 j-tiles into two
      persistent PSUM banks [1, 512].
    - Epilogue: density = acc * exp(-C||x_i||^2 - ln M) (norms via squared
      transposed x + ones-matvec so the factor lands in [1, 1024] layout).
"""
import math
from contextlib import ExitStack

import numpy as np

from concourse import bacc, mybir, tile
from concourse.bass_utils import run_bass_kernel_spmd
from concourse import masks

N, M, D = 8192, 8192, 128
NCORES = 8
NS = N // NCORES            # 1024 x-rows per core
P = 128                     # partitions
NT_X = NS // P              # 8 x tiles
NT_D = M // P               # 64 data tiles
NCHUNK = 8                  # data DMA chunks
TPC = NT_D // NCHUNK        # 8 tiles per chunk

C = 0.5 / math.sqrt(2.0 * math.pi)          # 0.19947114020071635
TWO_C = 2.0 * C                             # 0.3989422804014327
LNM = math.log(float(M))                    # ln 8192

F32 = mybir.dt.float32
F32R = mybir.dt.float32r
BF16 = mybir.dt.bfloat16

_CACHED_NC = None


def _build():
    nc = bacc.Bacc("TRN2", target_bir_lowering=False, debug=False)
    x_d = nc.dram_tensor("x", [NS, D], F32, kind="ExternalInput")
    d_d = nc.dram_tensor("data", [M, D], F32, kind="ExternalInput")
    o_d = nc.dram_tensor("out", [1, NS], F32, kind="ExternalOutput")

    # x loads contiguously (one 4KB packet per partition; row p*8+r lands at
    # [p, r]) — the induced permutation of xT columns is undone by one
    # on-chip reorder copy of the [1, 1024] result at the end.
    x_re = x_d.ap().rearrange("(p r) d -> p r d", p=P)     # [128, 8, 128]
    d_re = d_d.ap().rearrange("(s p) d -> p s d", p=P)     # [128, 64, 128]

    with tile.TileContext(nc) as tc, ExitStack() as ctx:
        const_pool = ctx.enter_context(tc.tile_pool(name="const", bufs=1))
        dT_pool = ctx.enter_context(tc.tile_pool(name="dT", bufs=1))
        xbuf_pool = ctx.enter_context(tc.tile_pool(name="xbuf", bufs=1))
        drow_pool = ctx.enter_context(tc.tile_pool(name="drow", bufs=4))
        scr_pool = ctx.enter_context(tc.tile_pool(name="scr", bufs=2))
        e_pool = ctx.enter_context(tc.tile_pool(name="e", bufs=3))
        out_pool = ctx.enter_context(tc.tile_pool(name="outp", bufs=1))
        ps_main = ctx.enter_context(tc.tile_pool(name="psm", bufs=2, space="PSUM"))
        ps_acc = ctx.enter_context(tc.tile_pool(name="psa", bufs=1, space="PSUM"))
        ps_tr = ctx.enter_context(tc.tile_pool(name="pst", bufs=2, space="PSUM"))

        ident = const_pool.tile([P, P], F32, tag="ident")
        masks.make_identity(nc, ident[:])
        ones_f = const_pool.tile([P, 1], F32, tag="onesf")
        nc.gpsimd.memset(ones_f[:], 1.0)
        ones_r = const_pool.tile([P, 1], F32R, tag="ones")
        nc.vector.tensor_copy(ones_r[:], ones_f[:])
        ones_b = const_pool.tile([P, 1], BF16, tag="onesb")
        nc.vector.tensor_copy(ones_b[:], ones_f[:])
        nlm_bias = const_pool.tile([1, 1], F32, tag="nlm")
        nc.gpsimd.memset(nlm_bias[:], -LNM)

        dataT = dT_pool.tile([P, M], F32R, tag="dataT")          # 32KB/part
        xT = xbuf_pool.tile([P, NS], F32R, tag="xT")
        xsqT = xbuf_pool.tile([P, NS], F32R, tag="xsqT")
        xrow = xbuf_pool.tile([P, NT_X, P], F32, tag="xrow")
        dnsq = const_pool.tile([P, NT_D], F32, tag="dnsq")
        dbias = const_pool.tile([P, NT_D], F32, tag="dbias")
        exf = out_pool.tile([1, NS], F32, tag="exf")
        dens = out_pool.tile([1, NS], F32, tag="dens")

        # ---- x prologue: load, transpose, squared-norm factor in [1, NS] ----
        # contiguous x is tiny (128 packets) — put it FIRST on the sync queue
        nc.sync.dma_start(xrow[:], x_re)
        for t in range(NT_X):
            tr = ps_tr.tile([P, P], F32, tag="tr")
            nc.tensor.transpose(tr[:], xrow[:, t, :], ident[:])
            nc.vector.tensor_copy(xT[:, t * P:(t + 1) * P], tr[:])
        nc.vector.tensor_mul(xsqT[:], xT[:].bitcast(F32), xT[:].bitcast(F32))
        pmx = ps_main.tile([P, NS], F32, tag="pm")
        for c2 in range(2):
            sl = slice(c2 * 512, (c2 + 1) * 512)
            nc.tensor.matmul(pmx[0:1, sl], ones_r[:], xsqT[:, sl],
                             start=True, stop=True)
        nc.scalar.activation(exf[:], pmx[0:1, :],
                             mybir.ActivationFunctionType.Exp,
                             bias=nlm_bias[:], scale=-C)
        # PE pre-warm: ~3.4us of continuous fp32 matmuls un-throttle the HAM
        # clock gate (1.2 -> 2.4GHz) before the main loop; output unused
        pmw = ps_main.tile([P, NS], F32, tag="pm")
        for w in range(2):
            nc.tensor.matmul(pmw[:, w * 512:(w + 1) * 512], ident[:],
                             xsqT[:, w * 512:(w + 1) * 512].bitcast(F32),
                             start=True, stop=True)

        # ---- data prologue: stream chunks; norms + transposes per tile ----
        for ch in range(NCHUNK):
            drow = drow_pool.tile([P, TPC, P], F32, tag="drow")
            nc.sync.dma_start(drow[:], d_re[:, ch * TPC:(ch + 1) * TPC, :])
            for k in range(TPC):
                s = ch * TPC + k
                scr = scr_pool.tile([P, P], F32, tag="scr")
                nc.vector.tensor_mul(scr[:], drow[:, k, :], drow[:, k, :])
                nc.vector.tensor_reduce(
                    dnsq[:, s:s + 1], scr[:],
                    axis=mybir.AxisListType.X, op=mybir.AluOpType.add)
                tr = ps_tr.tile([P, P], F32, tag="tr")
                nc.tensor.transpose(tr[:], drow[:, k, :], ident[:])
                nc.vector.tensor_copy(dataT[:, s * P:(s + 1) * P], tr[:])
            csl = slice(ch * TPC, (ch + 1) * TPC)
            nc.vector.tensor_scalar_mul(dbias[:, csl], dnsq[:, csl], -C)

        # ---- main loop over data tiles ----
        acc0 = ps_acc.tile([1, 512], F32, tag="acc0")
        acc1 = ps_acc.tile([1, 512], F32, tag="acc1")
        for jt in range(NT_D):
            pm = ps_main.tile([P, NS], F32, tag="pm")
            dsl = dataT[:, jt * P:(jt + 1) * P]
            nc.tensor.matmul(pm[:, 0:512], dsl, xT[:, 0:512],
                             start=True, stop=True)
            nc.tensor.matmul(pm[:, 512:1024], dsl, xT[:, 512:1024],
                             start=True, stop=True)
            e = e_pool.tile([P, NS], BF16, tag="e")
            nc.scalar.activation(e[:], pm[:],
                                 mybir.ActivationFunctionType.Exp,
                                 bias=dbias[:, jt:jt + 1], scale=TWO_C)
            nc.tensor.matmul(acc0[:], ones_b[:], e[:, 0:512],
                             start=(jt == 0), stop=(jt == NT_D - 1),
                             skip_group_check=True)
            nc.tensor.matmul(acc1[:], ones_b[:], e[:, 512:1024],
                             start=(jt == 0), stop=(jt == NT_D - 1),
                             skip_group_check=True)

        # ---- epilogue ----
        nc.vector.tensor_mul(dens[:, 0:512], acc0[:], exf[:, 0:512])
        nc.vector.tensor_mul(dens[:, 512:1024], acc1[:], exf[:, 512:1024])
        # undo the x row permutation: dens index r*128+p -> row 8p+r
        dens_o = out_pool.tile([1, NS], F32, tag="dens_o")
        nc.vector.tensor_copy(
            dens_o[:], dens[:].rearrange("o (r p) -> o p r", p=P))
        nc.sync.dma_start(o_d.ap(), dens_o[:])

    nc.compile()
    return nc


def kernel(x, data):
    global _CACHED_NC
    x = np.ascontiguousarray(np.asarray(x, dtype=np.float32))
    data = np.ascontiguousarray(np.asarray(data, dtype=np.float32))
    assert x.shape == (N, D) and data.shape == (M, D)

    if _CACHED_NC is None:
        _CACHED_NC = _build()
    nc = _CACHED_NC

    in_maps = [
        {"x": x[c * NS:(c + 1) * NS], "data": data} for c in range(NCORES)
    ]
    res = run_bass_kernel_spmd(nc, in_maps, list(range(NCORES)))
    dens = np.concatenate(
        [np.asarray(res.results[c]["out"]).reshape(NS) for c in range(NCORES)]
    )
    return dens.reshape(N, 1).astype(np.float32)


if __name__ == "__main__":
    rng = np.random.default_rng(0)
    x = rng.standard_normal((N, D), dtype=np.float32)
    data = rng.standard_normal((M, D), dtype=np.float32)
    out = kernel(x, data)
    print("kernel out", out.shape, out[:4, 0])

